# revision 22
# baseline (speedup 1.0000x reference)
"""Trainium2 Bass kernel for nn_BDRFuse (B,T,N,D = 8,64,196,768).

Per-core (pure data parallel over B): shard b -> core b.
Device pipeline per core, over n-chunks of NC=28 (7 chunks):
  phase 1: DMA x chunk in j-major (j,t) partition layout; PE-transpose
           128x128 tiles -> xT [d,tn']; fp32r matmul with U stationary
           -> v [8k, 1792 tn'] (masking deferred to phase 2).
  phase 2: SBUF DMA relayout v -> v2 [64t, (k*28+j)]; mask mult; PE
           matmuls over t-partitions for sum(v^2), DCT coeffs, mean;
           tanh via ACT; feats -> y = featsT.T @ W_dev (beta/softplus
           folded on host).
  phase 3: per 128-row tile, selector matmul replicates y rows into
           PSUM; one DVE scalar_tensor_tensor does
           h = x + mask_col * y_rep; DMA h out.
Host folds: U = proj_W.T, W_dev = perm(softplus(W_raw)) * sigmoid(beta),
DCT basis * 2.5, selector/mask auxiliaries. Returns (h, W) like the
reference.
"""

import os
from contextlib import ExitStack

import numpy as np

import concourse.bacc as bacc
import concourse.bass as bass
import concourse.mybir as mybir
import concourse.tile as tile
from concourse.bass_utils import run_bass_kernel_spmd

dt = mybir.dt
AF = mybir.ActivationFunctionType
ALU = mybir.AluOpType

B, T, N, D = 8, 64, 196, 768
K, P = 8, 2
BOUND, EPS = 2.5, 1e-6
NC = 28                      # n-chunk width
NCH = N // NC                # 7 chunks
JT = NC * T                  # 1792 rows (tn') per chunk
NT = JT // 128               # 14 tiles of 128 per chunk
DC = D // 128                # 6 d-chunks
FEAT = K * (P + 3)           # 40
# tn'-groups for phase 1 (tiles of 128 grouped for >=256-wide fp32r moving)
GROUPS = [(0, 4), (4, 4), (8, 4), (12, 2)]   # (first tile, n tiles)

FP32R = True                 # use fast fp32 matmul mode for big matmuls


def _f32r(ap):
    return ap.bitcast(dt.float32r) if FP32R else ap


def _host_consts(proj_W, W_raw, beta, valid_mask_b):
    """Host-folded constants. valid_mask_b is the per-core mask [T, N]."""
    c = {}
    c["U"] = np.ascontiguousarray(proj_W.T.astype(np.float32))        # [768, 8]

    # DCT-II basis exactly as reference._dct2_basis (float32)
    t = np.arange(T, dtype=np.float32) + np.float32(0.5)
    rows = [np.ones(T, dtype=np.float32)]
    for p in range(1, P + 1):
        rows.append(np.cos(np.float32(np.pi) * np.float32(p) * t / np.float32(T)).astype(np.float32))
    Bm = np.stack(rows, 0).astype(np.float32)                          # [3, 64]
    Bm = Bm / (np.sqrt((Bm * Bm).sum(1, keepdims=True)) + np.float32(EPS))
    ATc = np.zeros((T, P + 2), dtype=np.float32)                       # [64, 4]
    for p in range(P + 1):
        ATc[:, p] = np.float32(BOUND) * Bm[p]
    ATc[:, P + 1] = np.float32(BOUND) / np.float32(T)                  # mean row
    c["ATc"] = ATc

    # softplus(W_raw) like jax.nn.softplus = logaddexp(x, 0), f32
    W_sp = np.logaddexp(W_raw.astype(np.float32), np.float32(0.0)).astype(np.float32)  # [40, 768]
    c["W_out"] = W_sp
    sig_beta = (1.0 / (1.0 + np.exp(-beta.astype(np.float64)))).astype(np.float32)     # [768]
    W_dev = np.zeros((FEAT, D), dtype=np.float32)
    for k in range(K):
        for j in range(P + 3):
            W_dev[j * K + k] = W_sp[k * (P + 3) + j] * sig_beta
    c["W_dev"] = W_dev

    c["eps128"] = np.full((128, 1), EPS, dtype=np.float32)
    c["ones64"] = np.ones((T, 1), dtype=np.float32)
    c["onesrow"] = np.ones((1, T), dtype=np.float32)
    c["ident"] = np.eye(128, dtype=np.float32)

    # selector: repsel[j, tl*128+p] = 1 if j == 2*tl + p//64
    repsel = np.zeros((NC, NT * 128), dtype=np.float32)
    for tl in range(NT):
        for p in range(128):
            repsel[2 * tl + p // 64, tl * 128 + p] = 1.0
    c["repsel"] = repsel

    # per-core mask auxiliaries
    m = valid_mask_b.astype(np.float32)                                # [T, N]
    m_aux = np.zeros((NCH, NT * 128), dtype=np.float32)
    for ch in range(NCH):
        for tl in range(NT):
            for p in range(128):
                m_aux[ch, tl * 128 + p] = m[p % 64, ch * NC + 2 * tl + p // 64]
    c["m_aux"] = m_aux
    mask2 = np.zeros((NCH, T, K * NC), dtype=np.float32)
    for ch in range(NCH):
        blk = m[:, ch * NC:(ch + 1) * NC]                              # [64, 28]
        mask2[ch] = np.tile(blk, (1, K))
    c["mask2"] = mask2
    return c


def _emit(ctx, tc, io, reps=1):
    nc = tc.nc
    x_d, h_d = io["x"], io["h"]
    KN = K * NC  # 224

    const = ctx.enter_context(tc.tile_pool(name="const", bufs=1))
    sbx = ctx.enter_context(tc.tile_pool(name="sbx", bufs=2))
    sbxt = ctx.enter_context(tc.tile_pool(name="sbxt", bufs=2))
    sbv = ctx.enter_context(tc.tile_pool(name="sbv", bufs=2))
    sbs = ctx.enter_context(tc.tile_pool(name="sbs", bufs=2))
    sbh = ctx.enter_context(tc.tile_pool(name="sbh", bufs=4))
    pst = ctx.enter_context(tc.tile_pool(name="pst", bufs=2, space="PSUM"))
    psv = ctx.enter_context(tc.tile_pool(name="psv", bufs=1, space="PSUM"))
    pss = ctx.enter_context(tc.tile_pool(name="pss", bufs=3, space="PSUM"))
    psh = ctx.enter_context(tc.tile_pool(name="psh", bufs=2, space="PSUM"))

    # ---- constants into SBUF
    U_sb = const.tile([128, DC * K], dt.float32, tag="U")
    nc.sync.dma_start(U_sb[:], io["U"].rearrange("(c p) k -> p c k", p=128))
    AT_sb = const.tile([T, P + 2], dt.float32, tag="AT")
    nc.sync.dma_start(AT_sb[:], io["ATc"][:])
    ones64_sb = const.tile([T, 1], dt.float32, tag="o64")
    nc.sync.dma_start(ones64_sb[:], io["ones64"][:])
    onesrow_sb = const.tile([1, T], dt.float32, tag="orow")
    nc.sync.dma_start(onesrow_sb[:], io["onesrow"][:])
    eps_sb = const.tile([128, 1], dt.float32, tag="eps")
    nc.sync.dma_start(eps_sb[:], io["eps128"][:])
    ident_sb = const.tile([128, 128], dt.float32, tag="ident")
    nc.sync.dma_start(ident_sb[:], io["ident"][:])
    repsel_sb = const.tile([NC, NT * 128], dt.float32, tag="repsel")
    nc.sync.dma_start(repsel_sb[:], io["repsel"][:])
    W_sb = const.tile([FEAT, D], dt.float32, tag="W")
    nc.sync.dma_start(W_sb[:], io["W_dev"][:])
    U_r = const.tile([128, DC * K], dt.float32r, tag="Ur")
    nc.vector.tensor_copy(U_r[:], U_sb[:])
    W_r = const.tile([FEAT, D], dt.float32r, tag="Wr")
    nc.vector.tensor_copy(W_r[:], W_sb[:])
    repsel_r = const.tile([NC, NT * 128], dt.float32r, tag="repselr")
    nc.vector.tensor_copy(repsel_r[:], repsel_sb[:])

    for ch in range(NCH * reps):
        ch = ch % NCH
        n0 = ch * NC
        # ---- phase 1: x in (j-major: tn' = j*64 + t)
        x_sb = sbx.tile([128, NT * 768], dt.float32, tag="x")
        for j in range(NC):
            p0 = (j % 2) * 64
            tl = j // 2
            eng = nc.sync if j % 2 == 0 else nc.scalar
            eng.dma_start(
                x_sb[p0:p0 + 64, tl * 768:(tl + 1) * 768],
                x_d[:, n0 + j:n0 + j + 1, :].rearrange("t a d -> t (a d)"),
            )
        m_sb = sbs.tile([128, NT], dt.float32, tag="mcol")
        nc.sync.dma_start(
            m_sb[:],
            io["m_aux"][ch:ch + 1, :].rearrange("a (tl p) -> (a p) tl", p=128),
        )
        m2_sb = sbs.tile([T, KN], dt.float32, tag="m2")
        nc.sync.dma_start(
            m2_sb[:], io["mask2"][ch:ch + 1].rearrange("a t f -> (a t) f")
        )

        # ---- phase 1: transpose + projection, per tn'-group
        v_sb = sbv.tile([K, JT], dt.float32, tag="v")
        for gi, (t0, ntl) in enumerate(GROUPS):
            w = ntl * 128
            xt = sbxt.tile([128, DC * 512], dt.float32r, tag="xt")
            for dc in range(DC):
                ps_t = pst.tile([128, 512], dt.float32, tag="pst")
                for i in range(ntl):
                    tl = t0 + i
                    nc.tensor.transpose(
                        ps_t[:, i * 128:(i + 1) * 128],
                        x_sb[:, tl * 768 + dc * 128: tl * 768 + (dc + 1) * 128],
                        ident_sb[:],
                    )
                cp = nc.vector.tensor_copy if dc % 2 == 0 else nc.scalar.copy
                cp(xt[:, dc * 512:dc * 512 + w], ps_t[:, :w])
            v_ps = psv.tile([K, 512], dt.float32, tag="vps")
            for dc in range(DC):
                nc.tensor.matmul(
                    v_ps[:, :w],
                    lhsT=U_r[:, dc * K:(dc + 1) * K],
                    rhs=xt[:, dc * 512:dc * 512 + w],
                    start=(dc == 0),
                    stop=(dc == DC - 1),
                )
            nc.scalar.copy(v_sb[:, t0 * 128:t0 * 128 + w], v_ps[:, :w])

        # ---- phase 2: relayout v -> [t, (k,j)] via DRAM bounce, stats, feats, y
        vdr = io["v_dram"][ch:ch + 1].rearrange("a f -> (a f)")
        nc.scalar.dma_start(vdr.rearrange("(k f) -> k f", k=K), v_sb[:])
        v2 = sbs.tile([T, KN], dt.float32, tag="v2")
        nc.scalar.dma_start(
            v2[:], vdr.rearrange("(k j t) -> t k j", k=K, j=NC)
        )
        v2m = sbs.tile([T, KN], dt.float32, tag="v2m")
        nc.vector.tensor_tensor(v2m[:], v2[:], m2_sb[:], op=ALU.mult)
        sq = sbs.tile([T, KN], dt.float32, tag="sq")
        nc.scalar.square(sq[:], v2m[:])
        msum = pss.tile([1, KN], dt.float32, tag="pss")
        nc.tensor.matmul(msum[:], lhsT=ones64_sb[:], rhs=sq[:], start=True, stop=True)
        rms = sbs.tile([1, KN], dt.float32, tag="rms")
        nc.scalar.activation(rms[:], msum[:], AF.Sqrt, bias=eps_sb[:1, :], scale=1.0 / T)
        rmse = sbs.tile([1, KN], dt.float32, tag="rmse")
        nc.vector.tensor_scalar_add(rmse[:], rms[:], float(EPS))
        rinv = sbs.tile([1, KN], dt.float32, tag="rinv")
        nc.vector.reciprocal(rinv[:], rmse[:])
        rrep = pss.tile([T, KN], dt.float32, tag="pss")
        nc.tensor.matmul(rrep[:], lhsT=onesrow_sb[:], rhs=rinv[:], start=True, stop=True)
        arg = sbs.tile([T, KN], dt.float32, tag="arg")
        nc.vector.tensor_tensor(arg[:], v2m[:], rrep[:], op=ALU.mult)
        vb = sbs.tile([T, KN], dt.float32, tag="vb")
        nc.scalar.activation(vb[:], arg[:], AF.Tanh)
        sqb = sbs.tile([T, KN], dt.float32, tag="sqb")
        nc.scalar.square(sqb[:], vb[:])
        s1 = pss.tile([P + 2, KN], dt.float32, tag="pss")
        nc.tensor.matmul(s1[:], lhsT=AT_sb[:], rhs=vb[:], start=True, stop=True)
        s1_sb = sbs.tile([P + 2, KN], dt.float32, tag="s1sb")
        nc.vector.tensor_copy(s1_sb[:], s1[:])
        s2 = pss.tile([1, KN], dt.float32, tag="pss")
        nc.tensor.matmul(s2[:], lhsT=ones64_sb[:], rhs=sqb[:], start=True, stop=True)
        rmsf = sbs.tile([1, KN], dt.float32, tag="rmsf")
        nc.scalar.activation(
            rmsf[:], s2[:], AF.Sqrt, bias=eps_sb[:1, :], scale=float(BOUND * BOUND / T)
        )
        fdr = io["f_dram"][ch:ch + 1].rearrange("a f -> (a f)")
        nc.scalar.dma_start(
            fdr[: (P + 2) * KN].rearrange("(a f) -> a f", a=P + 2), s1_sb[:]
        )
        nc.scalar.dma_start(
            fdr[(P + 2) * KN:].rearrange("(a f) -> a f", a=1), rmsf[:]
        )
        featsT = sbs.tile([FEAT, NC], dt.float32, tag="feats")
        nc.scalar.dma_start(
            featsT[:], fdr.rearrange("(jj k j) -> (jj k) j", k=K, j=NC)
        )
        featsT_r = sbs.tile([FEAT, NC], dt.float32r, tag="featsr")
        nc.vector.tensor_copy(featsT_r[:], featsT[:])
        y_sb = sbs.tile([NC, D], dt.float32r, tag="y")
        for d0, dw in ((0, 512), (512, 256)):
            y_ps = pss.tile([NC, 512], dt.float32, tag="pss")
            nc.tensor.matmul(
                y_ps[:, :dw],
                lhsT=featsT_r[:],
                rhs=W_r[:, d0:d0 + dw],
                start=True,
                stop=True,
            )
            nc.scalar.copy(y_sb[:, d0:d0 + dw], y_ps[:, :dw])

        # ---- phase 3: y replicate via selector matmul + residual add
        for tl in range(NT):
            h_t = sbh.tile([128, 768], dt.float32, tag="h")
            for d0, dw in ((0, 512), (512, 256)):
                ph = psh.tile([128, 512], dt.float32, tag="psh")
                nc.tensor.matmul(
                    ph[:, :dw],
                    lhsT=repsel_r[:, tl * 128:(tl + 1) * 128],
                    rhs=y_sb[:, d0:d0 + dw],
                    start=True,
                    stop=True,
                )
                nc.vector.scalar_tensor_tensor(
                    h_t[:, d0:d0 + dw],
                    in0=ph[:, :dw],
                    scalar=m_sb[:, tl:tl + 1],
                    in1=x_sb[:, tl * 768 + d0:tl * 768 + d0 + dw],
                    op0=ALU.mult,
                    op1=ALU.add,
                )
            for half in range(2):
                j = 2 * tl + half
                eng = nc.sync if half == 0 else nc.scalar
                eng.dma_start(
                    h_d[:, n0 + j:n0 + j + 1, :].rearrange("t a d -> t (a d)"),
                    h_t[half * 64:(half + 1) * 64, :],
                )


def build_program(reps=1):
    nc = bacc.Bacc("TRN2", target_bir_lowering=False, debug=False)
    io = {}
    io["x"] = nc.dram_tensor("x", [T, N, D], dt.float32, kind="ExternalInput").ap()
    io["U"] = nc.dram_tensor("U", [D, K], dt.float32, kind="ExternalInput").ap()
    io["ATc"] = nc.dram_tensor("ATc", [T, P + 2], dt.float32, kind="ExternalInput").ap()
    io["ones64"] = nc.dram_tensor("ones64", [T, 1], dt.float32, kind="ExternalInput").ap()
    io["eps128"] = nc.dram_tensor("eps128", [128, 1], dt.float32, kind="ExternalInput").ap()
    io["onesrow"] = nc.dram_tensor("onesrow", [1, T], dt.float32, kind="ExternalInput").ap()
    io["ident"] = nc.dram_tensor("ident", [128, 128], dt.float32, kind="ExternalInput").ap()
    io["repsel"] = nc.dram_tensor("repsel", [NC, NT * 128], dt.float32, kind="ExternalInput").ap()
    io["W_dev"] = nc.dram_tensor("W_dev", [FEAT, D], dt.float32, kind="ExternalInput").ap()
    io["m_aux"] = nc.dram_tensor("m_aux", [NCH, NT * 128], dt.float32, kind="ExternalInput").ap()
    io["mask2"] = nc.dram_tensor("mask2", [NCH, T, K * NC], dt.float32, kind="ExternalInput").ap()
    io["h"] = nc.dram_tensor("h", [T, N, D], dt.float32, kind="ExternalOutput").ap()
    io["v_dram"] = nc.dram_tensor("v_dram", [NCH, K * JT], dt.float32).ap()
    io["f_dram"] = nc.dram_tensor("f_dram", [NCH, FEAT * NC], dt.float32).ap()
    with tile.TileContext(nc) as tc:
        with ExitStack() as ctx:
            _emit(ctx, tc, io, reps=reps)
    nc.compile()
    return nc


def make_in_maps(x, valid_mask, proj_W, W_raw, beta):
    in_maps = []
    W_out = None
    for b in range(B):
        c = _host_consts(proj_W, W_raw, beta, np.asarray(valid_mask)[b])
        W_out = c["W_out"]
        in_maps.append({
            "x": np.ascontiguousarray(np.asarray(x)[b], dtype=np.float32),
            "U": c["U"], "ATc": c["ATc"], "ones64": c["ones64"], "eps128": c["eps128"],
            "onesrow": c["onesrow"], "ident": c["ident"],
            "repsel": c["repsel"], "W_dev": c["W_dev"],
            "m_aux": c["m_aux"], "mask2": c["mask2"],
        })
    return in_maps, W_out


_NC_CACHE = {}


def _get_program():
    if "nc" not in _NC_CACHE:
        _NC_CACHE["nc"] = build_program()
    return _NC_CACHE["nc"]


def run_hw(x, valid_mask, proj_W, W_raw, beta, trace=False, **kw):
    nc = _get_program()
    in_maps, W_out = make_in_maps(x, valid_mask, proj_W, W_raw, beta)
    res = run_bass_kernel_spmd(nc, in_maps, core_ids=list(range(B)), trace=trace, **kw)
    h = np.stack([res.results[b]["h"] for b in range(B)], axis=0)
    return h, W_out, res


def kernel(x, valid_mask, proj_W, W_raw, beta):
    h, W_out, _ = run_hw(x, valid_mask, proj_W, W_raw, beta, trace=False)
    return h, W_out


# revision 37
# speedup vs baseline: 1.0324x; 1.0324x over previous
"""Trainium2 Bass kernel for nn_BDRFuse (B,T,N,D = 8,64,196,768).

Per-core (pure data parallel over B): shard b -> core b.
Device pipeline per core, over n-chunks of NC=14 (14 chunks):
  phase 1: 2 coalesced DMAs load the x chunk in j-major (j,t) partition
           layout [128p=(j%2)*64+t, (j//2)*768+d]; PE-transpose 128x128
           tiles -> xT [d, tn']; fp32r matmul with U stationary
           -> v [8k, 896 tn'].
  phase 2: DRAM-bounce relayout v -> v2 [64t, (k*14+j)]; mask mult; PE
           matmuls over t-partitions for sum(v^2), DCT coeffs, mean;
           tanh via ACT; feats -> y = featsT.T @ W_dev (beta/softplus
           folded on host).
  phase 3: per 128-row tile, selector matmul replicates y rows into
           PSUM; one DVE scalar_tensor_tensor does
           h = x + mask_col * y_rep into a whole-chunk h tile;
           2 coalesced DMAs store h.
Host folds: U = proj_W.T, W_dev = perm(softplus(W_raw)) * sigmoid(beta),
DCT basis * 2.5, selector/mask auxiliaries. Returns (h, W) like the
reference.
"""

from contextlib import ExitStack

import numpy as np

import concourse.bacc as bacc
import concourse.bass as bass
import concourse.mybir as mybir
import concourse.tile as tile
from concourse.bass_utils import run_bass_kernel_spmd

dt = mybir.dt
AF = mybir.ActivationFunctionType
ALU = mybir.AluOpType

B, T, N, D = 8, 64, 196, 768
K, P = 8, 2
BOUND, EPS = 2.5, 1e-6
NC = 14                      # n-chunk width
NCH = N // NC                # 14 chunks
JT = NC * T                  # 896 rows (tn') per chunk
NT = JT // 128               # 7 tiles of 128 per chunk
DC = D // 128                # 6 d-chunks
FEAT = K * (P + 3)           # 40
KN = K * NC                  # 112
# tn'-groups for phase 1 (tiles of 128 grouped for >=256-wide fp32r moving)
GROUPS = [(0, 4), (4, 3)]    # (first tile, n tiles) -> widths 512, 384


def _host_consts(proj_W, W_raw, beta, valid_mask_b):
    """Host-folded constants. valid_mask_b is the per-core mask [T, N]."""
    c = {}
    c["U"] = np.ascontiguousarray(proj_W.T.astype(np.float32))        # [768, 8]

    # DCT-II basis exactly as reference._dct2_basis (float32)
    t = np.arange(T, dtype=np.float32) + np.float32(0.5)
    rows = [np.ones(T, dtype=np.float32)]
    for p in range(1, P + 1):
        rows.append(np.cos(np.float32(np.pi) * np.float32(p) * t / np.float32(T)).astype(np.float32))
    Bm = np.stack(rows, 0).astype(np.float32)                          # [3, 64]
    Bm = Bm / (np.sqrt((Bm * Bm).sum(1, keepdims=True)) + np.float32(EPS))
    ATc = np.zeros((T, P + 2), dtype=np.float32)                       # [64, 4]
    for p in range(P + 1):
        ATc[:, p] = np.float32(BOUND) * Bm[p]
    ATc[:, P + 1] = np.float32(BOUND) / np.float32(T)                  # mean row
    c["ATc"] = ATc

    # softplus(W_raw) like jax.nn.softplus = logaddexp(x, 0), f32
    W_sp = np.logaddexp(W_raw.astype(np.float32), np.float32(0.0)).astype(np.float32)
    c["W_out"] = W_sp
    sig_beta = (1.0 / (1.0 + np.exp(-beta.astype(np.float64)))).astype(np.float32)
    W_dev = np.zeros((FEAT, D), dtype=np.float32)
    for k in range(K):
        for j in range(P + 3):
            W_dev[j * K + k] = W_sp[k * (P + 3) + j] * sig_beta
    c["W_dev"] = W_dev

    c["eps128"] = np.full((128, 1), EPS, dtype=np.float32)
    c["ones64"] = np.ones((T, 1), dtype=np.float32)
    c["onesrow"] = np.ones((1, T), dtype=np.float32)
    c["ident"] = np.eye(128, dtype=np.float32)

    # selector: repsel[j, tl*128+p] = 1 if j == 2*tl + p//64  (global, per-chunk local j)
    repsel = np.zeros((NC, NT * 128), dtype=np.float32)
    for tl in range(NT):
        for p in range(128):
            repsel[2 * tl + p // 64, tl * 128 + p] = 1.0
    c["repsel"] = repsel

    # per-core mask auxiliaries (one-time loads)
    m = valid_mask_b.astype(np.float32)                                # [T, N]
    # m_all[p, ch*NT+tl] = mask[t(p), n(ch, tl, p)]
    m_all = np.zeros((128, NCH * NT), dtype=np.float32)
    for ch in range(NCH):
        for tl in range(NT):
            for p in range(128):
                m_all[p, ch * NT + tl] = m[p % 64, ch * NC + 2 * tl + p // 64]
    c["m_all"] = m_all
    # mask2_all[t, ch*KN + k*NC + j] = mask[t, ch*NC + j]
    mask2 = np.zeros((T, NCH * KN), dtype=np.float32)
    for ch in range(NCH):
        blk = m[:, ch * NC:(ch + 1) * NC]                              # [64, 14]
        mask2[:, ch * KN:(ch + 1) * KN] = np.tile(blk, (1, K))
    c["mask2"] = mask2
    return c


def _emit(ctx, tc, io, reps=1):
    nc = tc.nc
    x_d, h_d = io["x"], io["h"]

    const = ctx.enter_context(tc.tile_pool(name="const", bufs=1))
    sbx = ctx.enter_context(tc.tile_pool(name="sbx", bufs=4))
    sbxt = ctx.enter_context(tc.tile_pool(name="sbxt", bufs=1))
    sbv = ctx.enter_context(tc.tile_pool(name="sbv", bufs=2))
    sbs = ctx.enter_context(tc.tile_pool(name="sbs", bufs=2))
    sbh = ctx.enter_context(tc.tile_pool(name="sbh", bufs=2))
    sby = ctx.enter_context(tc.tile_pool(name="sby", bufs=4))
    pst = ctx.enter_context(tc.tile_pool(name="pst", bufs=2, space="PSUM"))
    psv = ctx.enter_context(tc.tile_pool(name="psv", bufs=1, space="PSUM"))
    pss = ctx.enter_context(tc.tile_pool(name="pss", bufs=2, space="PSUM"))
    psq = ctx.enter_context(tc.tile_pool(name="psq", bufs=1, space="PSUM"))
    psh = ctx.enter_context(tc.tile_pool(name="psh", bufs=2, space="PSUM"))

    # ---- constants into SBUF
    U_sb = const.tile([128, DC * K], dt.float32, tag="U")
    nc.sync.dma_start(U_sb[:], io["U"].rearrange("(c p) k -> p c k", p=128))
    AT_sb = const.tile([T, P + 2], dt.float32, tag="AT")
    nc.sync.dma_start(AT_sb[:], io["ATc"][:])
    ones64_sb = const.tile([T, 1], dt.float32, tag="o64")
    nc.sync.dma_start(ones64_sb[:], io["ones64"][:])
    onesrow_sb = const.tile([1, T], dt.float32, tag="orow")
    nc.sync.dma_start(onesrow_sb[:], io["onesrow"][:])
    eps_sb = const.tile([128, 1], dt.float32, tag="eps")
    nc.sync.dma_start(eps_sb[:], io["eps128"][:])
    ident_sb = const.tile([128, 128], dt.float32, tag="ident")
    nc.sync.dma_start(ident_sb[:], io["ident"][:])
    m_all = const.tile([128, NCH * NT], dt.float32, tag="mall")
    nc.sync.dma_start(m_all[:], io["m_all"][:])
    mask2_all = const.tile([T, NCH * KN], dt.float32, tag="mask2")
    nc.sync.dma_start(mask2_all[:], io["mask2"][:])
    # f32 staging for f32r consts shares the xt tag (transient slots)
    repsel_f = sbxt.tile([NC, NT * 128], dt.float32, tag="xt")
    nc.sync.dma_start(repsel_f[:], io["repsel"][:])
    repsel_r = const.tile([NC, NT * 128], dt.float32r, tag="repselr")
    nc.vector.tensor_copy(repsel_r[:], repsel_f[:])
    W_f = sbxt.tile([FEAT, D], dt.float32, tag="xt")
    nc.sync.dma_start(W_f[:], io["W_dev"][:])
    W_r = const.tile([FEAT, D], dt.float32r, tag="Wr")
    nc.vector.tensor_copy(W_r[:], W_f[:])
    U_r = const.tile([128, DC * K], dt.float32r, tag="Ur")
    nc.vector.tensor_copy(U_r[:], U_sb[:])

    def phase1(ch):
        """x in + transpose + projection -> v_sb. Returns live tiles."""
        n0 = ch * NC
        x_parts = (sbx.tile([128, 4 * 768], dt.float32, tag="xa", name="xa"),
                   sbx.tile([128, 3 * 768], dt.float32, tag="xb", name="xb"))
        for half in range(2):
            for pi, (ta, tb) in enumerate(((0, 4), (4, 7))):
                nc.sync.dma_start(
                    x_parts[pi][half * 64:(half + 1) * 64, :].rearrange(
                        "p (tl d) -> p tl d", d=768
                    ),
                    x_d[:, n0 + half + 2 * ta:n0 + half + 2 * tb - 1:2, :],
                )
        v_sb = sbv.tile([K, JT], dt.float32, tag="v")
        for gi, (t0, ntl) in enumerate(GROUPS):
            w = ntl * 128
            xp = x_parts[gi]
            xt = sbxt.tile([128, DC * 512], dt.float32r, tag="xt")
            for dc in range(DC):
                ps_t = pst.tile([128, 512], dt.float32, tag="pst")
                for i in range(ntl):
                    nc.tensor.transpose(
                        ps_t[:, i * 128:(i + 1) * 128],
                        xp[:, i * 768 + dc * 128: i * 768 + (dc + 1) * 128],
                        ident_sb[:],
                    )
                cp = nc.vector.tensor_copy if dc % 2 == 0 else nc.scalar.copy
                cp(xt[:, dc * 512:dc * 512 + w], ps_t[:, :w])
            v_ps = psv.tile([K, 512], dt.float32, tag="vps")
            for dc in range(DC):
                nc.tensor.matmul(
                    v_ps[:, :w],
                    lhsT=U_r[:, dc * K:(dc + 1) * K],
                    rhs=xt[:, dc * 512:dc * 512 + w],
                    start=(dc == 0),
                    stop=(dc == DC - 1),
                )
            nc.scalar.copy(v_sb[:, t0 * 128:t0 * 128 + w], v_ps[:, :w])
        return {"x": x_parts, "v": v_sb}

    def phase2(ch, st):
        """v relayout + stats + feats + y."""
        v_sb = st["v"]
        v2t = psq.tile([T, KN], dt.float32, tag="v2t")
        for j in range(NC):
            nc.tensor.transpose(
                bass.AP(v2t[:].tensor, v2t[:].offset + j, [[v2t[:].ap[0][0], T], [NC, K]]),
                v_sb[:, j * T:(j + 1) * T],
                ident_sb[:K, :K],
            )
        v2 = sbs.tile([T, KN], dt.float32, tag="v2")
        nc.vector.tensor_tensor(
            v2[:], v2t[:], mask2_all[:, ch * KN:(ch + 1) * KN], op=ALU.mult
        )
        sq = sbs.tile([T, KN], dt.float32, tag="sq")
        nc.vector.tensor_tensor(sq[:], v2[:], v2[:], op=ALU.mult)
        msum = pss.tile([1, KN], dt.float32, tag="pss")
        nc.tensor.matmul(msum[:], lhsT=ones64_sb[:], rhs=sq[:], start=True, stop=True)
        rms = sbs.tile([1, KN], dt.float32, tag="rms")
        nc.scalar.activation(rms[:], msum[:], AF.Sqrt, bias=eps_sb[:1, :], scale=1.0 / T)
        rmse = sbs.tile([1, KN], dt.float32, tag="rmse")
        nc.vector.tensor_scalar_add(rmse[:], rms[:], float(EPS))
        rinv = sbs.tile([1, KN], dt.float32, tag="rinv")
        nc.vector.reciprocal(rinv[:], rmse[:])
        rrep = pss.tile([T, KN], dt.float32, tag="pss")
        nc.tensor.matmul(rrep[:], lhsT=onesrow_sb[:], rhs=rinv[:], start=True, stop=True)
        arg = sbs.tile([T, KN], dt.float32, tag="arg")
        nc.vector.tensor_tensor(arg[:], v2[:], rrep[:], op=ALU.mult)
        vb = sbs.tile([T, KN], dt.float32, tag="vb")
        nc.scalar.activation(vb[:], arg[:], AF.Tanh)
        sqb = sbs.tile([T, KN], dt.float32, tag="sqb")
        nc.vector.tensor_tensor(sqb[:], vb[:], vb[:], op=ALU.mult)
        s1_sb = sbs.tile([P + 2, KN], dt.float32, tag="s1sb")
        s1 = pss.tile([P + 2, KN], dt.float32, tag="pss")
        nc.tensor.matmul(s1[:], lhsT=AT_sb[:], rhs=vb[:], start=True, stop=True)
        nc.scalar.copy(s1_sb[:], s1[:])
        s2 = pss.tile([1, KN], dt.float32, tag="pss")
        nc.tensor.matmul(s2[:], lhsT=ones64_sb[:], rhs=sqb[:], start=True, stop=True)
        rmsf = sbs.tile([1, KN], dt.float32, tag="rmsf")
        nc.scalar.activation(
            rmsf[:], s2[:], AF.Sqrt,
            bias=eps_sb[:1, :], scale=float(BOUND * BOUND / T),
        )
        fdr = io["f_dram"][ch:ch + 1].rearrange("a f -> (a f)")
        nc.scalar.dma_start(
            fdr[:(P + 2) * KN].rearrange("(a f) -> a f", a=P + 2), s1_sb[:]
        )
        nc.scalar.dma_start(
            fdr[(P + 2) * KN:].rearrange("(a f) -> a f", a=1), rmsf[:]
        )
        featsT = sbs.tile([FEAT, NC], dt.float32, tag="feats")
        nc.scalar.dma_start(
            featsT[:], fdr.rearrange("(jj k j) -> (jj k) j", k=K, j=NC)
        )
        featsT_r = sbs.tile([FEAT, NC], dt.float32r, tag="featsr")
        nc.vector.tensor_copy(featsT_r[:], featsT[:])
        y_sb = sby.tile([NC, D], dt.float32r, tag="y")
        for d0, dw in ((0, 512), (512, 256)):
            y_ps = pss.tile([NC, 512], dt.float32, tag="pss")
            nc.tensor.matmul(
                y_ps[:, :dw],
                lhsT=featsT_r[:],
                rhs=W_r[:, d0:d0 + dw],
                start=True,
                stop=True,
            )
            nc.scalar.copy(y_sb[:, d0:d0 + dw], y_ps[:, :dw])
        st["y"] = y_sb

    def phase3(ch, st):
        """y replicate via selector matmul + residual add + h out."""
        n0 = ch * NC
        x_parts, y_sb = st["x"], st["y"]
        h_parts = (sbh.tile([128, 4 * 768], dt.float32, tag="ha", name="ha"),
                   sbh.tile([128, 3 * 768], dt.float32, tag="hb", name="hb"))
        for pi, (ta, tb) in enumerate(((0, 4), (4, 7))):
            xp, hp = x_parts[pi], h_parts[pi]
            for i in range(tb - ta):
                tl = ta + i
                m_col = m_all[:, ch * NT + tl:ch * NT + tl + 1]
                for d0, dw in ((0, 512), (512, 256)):
                    ph = psh.tile([128, 512], dt.float32, tag="psh")
                    nc.tensor.matmul(
                        ph[:, :dw],
                        lhsT=repsel_r[:, tl * 128:(tl + 1) * 128],
                        rhs=y_sb[:, d0:d0 + dw],
                        start=True,
                        stop=True,
                    )
                    nc.vector.scalar_tensor_tensor(
                        hp[:, i * 768 + d0:i * 768 + d0 + dw],
                        in0=ph[:, :dw],
                        scalar=m_col,
                        in1=xp[:, i * 768 + d0:i * 768 + d0 + dw],
                        op0=ALU.mult,
                        op1=ALU.add,
                    )
            for half in range(2):
                nc.gpsimd.dma_start(
                    h_d[:, n0 + half + 2 * ta:n0 + half + 2 * tb - 1:2, :],
                    hp[half * 64:(half + 1) * 64, :].rearrange(
                        "p (tl d) -> p tl d", d=768
                    ),
                )

    # software-pipelined emission: phase1(it) | phase2(it-1) | phase3(it-2)
    total = NCH * reps
    states = {}
    for it in range(total + 3):
        if it >= 3:
            phase3((it - 3) % NCH, states[it - 3])
            del states[it - 3]
        if 1 <= it <= total:
            phase2((it - 1) % NCH, states[it - 1])
        if it < total:
            states[it] = phase1(it % NCH)


def build_program(reps=1):
    nc = bacc.Bacc("TRN2", target_bir_lowering=False, debug=False)
    io = {}
    io["x"] = nc.dram_tensor("x", [T, N, D], dt.float32, kind="ExternalInput").ap()
    io["U"] = nc.dram_tensor("U", [D, K], dt.float32, kind="ExternalInput").ap()
    io["ATc"] = nc.dram_tensor("ATc", [T, P + 2], dt.float32, kind="ExternalInput").ap()
    io["ones64"] = nc.dram_tensor("ones64", [T, 1], dt.float32, kind="ExternalInput").ap()
    io["eps128"] = nc.dram_tensor("eps128", [128, 1], dt.float32, kind="ExternalInput").ap()
    io["onesrow"] = nc.dram_tensor("onesrow", [1, T], dt.float32, kind="ExternalInput").ap()
    io["ident"] = nc.dram_tensor("ident", [128, 128], dt.float32, kind="ExternalInput").ap()
    io["repsel"] = nc.dram_tensor("repsel", [NC, NT * 128], dt.float32, kind="ExternalInput").ap()
    io["W_dev"] = nc.dram_tensor("W_dev", [FEAT, D], dt.float32, kind="ExternalInput").ap()
    io["m_all"] = nc.dram_tensor("m_all", [128, NCH * NT], dt.float32, kind="ExternalInput").ap()
    io["mask2"] = nc.dram_tensor("mask2", [T, NCH * KN], dt.float32, kind="ExternalInput").ap()
    io["h"] = nc.dram_tensor("h", [T, N, D], dt.float32, kind="ExternalOutput").ap()
    io["v_dram"] = nc.dram_tensor("v_dram", [NCH, K * JT], dt.float32).ap()
    io["f_dram"] = nc.dram_tensor("f_dram", [NCH, (P + 3) * KN], dt.float32).ap()
    with tile.TileContext(nc) as tc:
        with ExitStack() as ctx:
            _emit(ctx, tc, io, reps=reps)
    nc.compile()
    return nc


def make_in_maps(x, valid_mask, proj_W, W_raw, beta):
    in_maps = []
    W_out = None
    for b in range(B):
        c = _host_consts(proj_W, W_raw, beta, np.asarray(valid_mask)[b])
        W_out = c["W_out"]
        in_maps.append({
            "x": np.ascontiguousarray(np.asarray(x)[b], dtype=np.float32),
            "U": c["U"], "ATc": c["ATc"], "ones64": c["ones64"], "eps128": c["eps128"],
            "onesrow": c["onesrow"], "ident": c["ident"],
            "repsel": c["repsel"], "W_dev": c["W_dev"],
            "m_all": c["m_all"], "mask2": c["mask2"],
        })
    return in_maps, W_out


_NC_CACHE = {}


def _get_program():
    if "nc" not in _NC_CACHE:
        _NC_CACHE["nc"] = build_program()
    return _NC_CACHE["nc"]


def run_hw(x, valid_mask, proj_W, W_raw, beta, trace=False, **kw):
    nc = _get_program()
    in_maps, W_out = make_in_maps(x, valid_mask, proj_W, W_raw, beta)
    res = run_bass_kernel_spmd(nc, in_maps, core_ids=list(range(B)), trace=trace, **kw)
    h = np.stack([res.results[b]["h"] for b in range(B)], axis=0)
    return h, W_out, res


def kernel(x, valid_mask, proj_W, W_raw, beta):
    h, W_out, _ = run_hw(x, valid_mask, proj_W, W_raw, beta, trace=False)
    return h, W_out


# revision 41
# speedup vs baseline: 55051.9944x; 53325.1403x over previous
"""Trainium2 Bass kernel for nn_BDRFuse (B,T,N,D = 8,64,196,768).

Per-core (pure data parallel over B): shard b -> core b.
Device pipeline per core, over n-chunks of NC=14 (14 chunks):
  phase 1: 2 coalesced DMAs load the x chunk in j-major (j,t) partition
           layout [128p=(j%2)*64+t, (j//2)*768+d]; PE-transpose 128x128
           tiles -> xT [d, tn']; fp32r matmul with U stationary
           -> v [8k, 896 tn'].
  phase 2: DRAM-bounce relayout v -> v2 [64t, (k*14+j)]; mask mult; PE
           matmuls over t-partitions for sum(v^2), DCT coeffs, mean;
           tanh via ACT; feats -> y = featsT.T @ W_dev (beta/softplus
           folded on host).
  phase 3: per 128-row tile, selector matmul replicates y rows into
           PSUM; one DVE scalar_tensor_tensor does
           h = x + mask_col * y_rep into a whole-chunk h tile;
           2 coalesced DMAs store h.
Host folds: U = proj_W.T, W_dev = perm(softplus(W_raw)) * sigmoid(beta),
DCT basis * 2.5, selector/mask auxiliaries. Returns (h, W) like the
reference.
"""

from contextlib import ExitStack

import numpy as np

import concourse.bacc as bacc
import concourse.bass as bass
import concourse.mybir as mybir
import concourse.tile as tile
from concourse.bass_utils import run_bass_kernel_spmd

dt = mybir.dt
AF = mybir.ActivationFunctionType
ALU = mybir.AluOpType

B, T, N, D = 8, 64, 196, 768
K, P = 8, 2
BOUND, EPS = 2.5, 1e-6
NC = 14                      # n-chunk width
NCH = N // NC                # 14 chunks
JT = NC * T                  # 896 rows (tn') per chunk
NT = JT // 128               # 7 tiles of 128 per chunk
DC = D // 128                # 6 d-chunks
FEAT = K * (P + 3)           # 40
KN = K * NC                  # 112
# tn'-groups for phase 1 (tiles of 128 grouped for >=256-wide fp32r moving)
GROUPS = [(0, 4), (4, 3)]    # (first tile, n tiles) -> widths 512, 384


def _host_consts(proj_W, W_raw, beta, valid_mask_b):
    """Host-folded constants. valid_mask_b is the per-core mask [T, N]."""
    c = {}
    c["U"] = np.ascontiguousarray(proj_W.T.astype(np.float32))        # [768, 8]

    # DCT-II basis exactly as reference._dct2_basis (float32)
    t = np.arange(T, dtype=np.float32) + np.float32(0.5)
    rows = [np.ones(T, dtype=np.float32)]
    for p in range(1, P + 1):
        rows.append(np.cos(np.float32(np.pi) * np.float32(p) * t / np.float32(T)).astype(np.float32))
    Bm = np.stack(rows, 0).astype(np.float32)                          # [3, 64]
    Bm = Bm / (np.sqrt((Bm * Bm).sum(1, keepdims=True)) + np.float32(EPS))
    ATc = np.zeros((T, P + 2), dtype=np.float32)                       # [64, 4]
    for p in range(P + 1):
        ATc[:, p] = np.float32(BOUND) * Bm[p]
    ATc[:, P + 1] = np.float32(BOUND) / np.float32(T)                  # mean row
    c["ATc"] = ATc

    # softplus(W_raw) like jax.nn.softplus = logaddexp(x, 0), f32
    W_sp = np.logaddexp(W_raw.astype(np.float32), np.float32(0.0)).astype(np.float32)
    c["W_out"] = W_sp
    sig_beta = (1.0 / (1.0 + np.exp(-beta.astype(np.float64)))).astype(np.float32)
    W_dev = np.zeros((FEAT, D), dtype=np.float32)
    for k in range(K):
        for j in range(P + 3):
            W_dev[j * K + k] = W_sp[k * (P + 3) + j] * sig_beta
    c["W_dev"] = W_dev

    c["eps128"] = np.full((128, 1), EPS, dtype=np.float32)
    c["ones64"] = np.ones((T, 1), dtype=np.float32)
    c["onesrow"] = np.ones((1, T), dtype=np.float32)
    c["ident"] = np.eye(128, dtype=np.float32)

    # selector: repsel[j, tl*128+p] = 1 if j == 2*tl + p//64  (global, per-chunk local j)
    repsel = np.zeros((NC, NT * 128), dtype=np.float32)
    for tl in range(NT):
        for p in range(128):
            repsel[2 * tl + p // 64, tl * 128 + p] = 1.0
    c["repsel"] = repsel

    # per-core mask auxiliaries (one-time loads)
    m = valid_mask_b.astype(np.float32)                                # [T, N]
    # m_all[p, ch*NT+tl] = mask[t(p), n(ch, tl, p)]
    m_all = np.zeros((128, NCH * NT), dtype=np.float32)
    for ch in range(NCH):
        for tl in range(NT):
            for p in range(128):
                m_all[p, ch * NT + tl] = m[p % 64, ch * NC + 2 * tl + p // 64]
    c["m_all"] = m_all
    # mask2_all[t, ch*KN + k*NC + j] = mask[t, ch*NC + j]
    mask2 = np.zeros((T, NCH * KN), dtype=np.float32)
    for ch in range(NCH):
        blk = m[:, ch * NC:(ch + 1) * NC]                              # [64, 14]
        mask2[:, ch * KN:(ch + 1) * KN] = np.tile(blk, (1, K))
    c["mask2"] = mask2
    return c


def _emit(ctx, tc, io, reps=1):
    nc = tc.nc
    x_d, h_d = io["x"], io["h"]

    const = ctx.enter_context(tc.tile_pool(name="const", bufs=1))
    sbx = ctx.enter_context(tc.tile_pool(name="sbx", bufs=4))
    sbxt = ctx.enter_context(tc.tile_pool(name="sbxt", bufs=1))
    sbv = ctx.enter_context(tc.tile_pool(name="sbv", bufs=2))
    sbs = ctx.enter_context(tc.tile_pool(name="sbs", bufs=2))
    sbh = ctx.enter_context(tc.tile_pool(name="sbh", bufs=2))
    sby = ctx.enter_context(tc.tile_pool(name="sby", bufs=4))
    pst = ctx.enter_context(tc.tile_pool(name="pst", bufs=2, space="PSUM"))
    psv = ctx.enter_context(tc.tile_pool(name="psv", bufs=1, space="PSUM"))
    pss = ctx.enter_context(tc.tile_pool(name="pss", bufs=2, space="PSUM"))
    psq = ctx.enter_context(tc.tile_pool(name="psq", bufs=1, space="PSUM"))
    psh = ctx.enter_context(tc.tile_pool(name="psh", bufs=2, space="PSUM"))

    # ---- constants into SBUF
    U_sb = const.tile([128, DC * K], dt.float32, tag="U")
    nc.sync.dma_start(U_sb[:], io["U"].rearrange("(c p) k -> p c k", p=128))
    AT_sb = const.tile([T, P + 2], dt.float32, tag="AT")
    nc.sync.dma_start(AT_sb[:], io["ATc"][:])
    ones64_sb = const.tile([T, 1], dt.float32, tag="o64")
    nc.sync.dma_start(ones64_sb[:], io["ones64"][:])
    onesrow_sb = const.tile([1, T], dt.float32, tag="orow")
    nc.sync.dma_start(onesrow_sb[:], io["onesrow"][:])
    eps_sb = const.tile([128, 1], dt.float32, tag="eps")
    nc.sync.dma_start(eps_sb[:], io["eps128"][:])
    ident_sb = const.tile([128, 128], dt.float32, tag="ident")
    nc.sync.dma_start(ident_sb[:], io["ident"][:])
    m_all = const.tile([128, NCH * NT], dt.float32, tag="mall")
    nc.sync.dma_start(m_all[:], io["m_all"][:])
    mask2_all = const.tile([T, NCH * KN], dt.float32, tag="mask2")
    nc.sync.dma_start(mask2_all[:], io["mask2"][:])
    # f32 staging for f32r consts shares the xt tag (transient slots)
    repsel_f = sbxt.tile([NC, NT * 128], dt.float32, tag="xt")
    nc.sync.dma_start(repsel_f[:], io["repsel"][:])
    repsel_r = const.tile([NC, NT * 128], dt.float32r, tag="repselr")
    nc.vector.tensor_copy(repsel_r[:], repsel_f[:])
    W_f = sbxt.tile([FEAT, D], dt.float32, tag="xt")
    nc.sync.dma_start(W_f[:], io["W_dev"][:])
    W_r = const.tile([FEAT, D], dt.float32r, tag="Wr")
    nc.vector.tensor_copy(W_r[:], W_f[:])
    U_r = const.tile([128, DC * K], dt.float32r, tag="Ur")
    nc.vector.tensor_copy(U_r[:], U_sb[:])

    def phase1(ch):
        """x in + transpose + projection -> v_sb. Returns live tiles."""
        n0 = ch * NC
        x_parts = (sbx.tile([128, 4 * 768], dt.float32, tag="xa", name="xa"),
                   sbx.tile([128, 3 * 768], dt.float32, tag="xb", name="xb"))
        for pi, (ta, tb) in enumerate(((0, 4), (4, 7))):
            for half in range(2):
                nc.sync.dma_start(
                    x_parts[pi][half * 64:(half + 1) * 64, :].rearrange(
                        "p (tl d) -> p tl d", d=768
                    ),
                    x_d[:, n0 + half + 2 * ta:n0 + half + 2 * tb - 1:2, :],
                )
        v_sb = sbv.tile([K, JT], dt.float32, tag="v")
        for gi, (t0, ntl) in enumerate(GROUPS):
            w = ntl * 128
            xp = x_parts[gi]
            xt = sbxt.tile([128, DC * 512], dt.float32r, tag="xt")
            for dc in range(DC):
                ps_t = pst.tile([128, 512], dt.float32, tag="pst")
                for i in range(ntl):
                    nc.tensor.transpose(
                        ps_t[:, i * 128:(i + 1) * 128],
                        xp[:, i * 768 + dc * 128: i * 768 + (dc + 1) * 128],
                        ident_sb[:],
                    )
                cp = nc.vector.tensor_copy if dc % 2 == 0 else nc.scalar.copy
                cp(xt[:, dc * 512:dc * 512 + w], ps_t[:, :w])
            v_ps = psv.tile([K, 512], dt.float32, tag="vps")
            for dc in range(DC):
                nc.tensor.matmul(
                    v_ps[:, :w],
                    lhsT=U_r[:, dc * K:(dc + 1) * K],
                    rhs=xt[:, dc * 512:dc * 512 + w],
                    start=(dc == 0),
                    stop=(dc == DC - 1),
                )
            nc.scalar.copy(v_sb[:, t0 * 128:t0 * 128 + w], v_ps[:, :w])
        return {"x": x_parts, "v": v_sb}

    def phase2(ch, st):
        """v relayout + stats + feats + y."""
        v_sb = st["v"]
        v2t = psq.tile([T, KN], dt.float32, tag="v2t")
        for j in range(NC):
            nc.tensor.transpose(
                bass.AP(v2t[:].tensor, v2t[:].offset + j, [[v2t[:].ap[0][0], T], [NC, K]]),
                v_sb[:, j * T:(j + 1) * T],
                ident_sb[:K, :K],
            )
        v2 = sbs.tile([T, KN], dt.float32, tag="v2")
        nc.vector.tensor_tensor(
            v2[:], v2t[:], mask2_all[:, ch * KN:(ch + 1) * KN], op=ALU.mult
        )
        sq = sbs.tile([T, KN], dt.float32, tag="sq")
        nc.vector.tensor_tensor(sq[:], v2[:], v2[:], op=ALU.mult)
        msum = pss.tile([1, KN], dt.float32, tag="pss")
        nc.tensor.matmul(msum[:], lhsT=ones64_sb[:], rhs=sq[:], start=True, stop=True)
        rms = sbs.tile([1, KN], dt.float32, tag="rms")
        nc.scalar.activation(rms[:], msum[:], AF.Sqrt, bias=eps_sb[:1, :], scale=1.0 / T)
        rmse = sbs.tile([1, KN], dt.float32, tag="rmse")
        nc.vector.tensor_scalar_add(rmse[:], rms[:], float(EPS))
        rinv = sbs.tile([1, KN], dt.float32, tag="rinv")
        nc.vector.reciprocal(rinv[:], rmse[:])
        rrep = pss.tile([T, KN], dt.float32, tag="pss")
        nc.tensor.matmul(rrep[:], lhsT=onesrow_sb[:], rhs=rinv[:], start=True, stop=True)
        arg = sbs.tile([T, KN], dt.float32, tag="arg")
        nc.vector.tensor_tensor(arg[:], v2[:], rrep[:], op=ALU.mult)
        vb = sbs.tile([T, KN], dt.float32, tag="vb")
        nc.scalar.activation(vb[:], arg[:], AF.Tanh)
        sqb = sbs.tile([T, KN], dt.float32, tag="sqb")
        nc.vector.tensor_tensor(sqb[:], vb[:], vb[:], op=ALU.mult)
        s1_sb = sbs.tile([P + 2, KN], dt.float32, tag="s1sb")
        s1 = pss.tile([P + 2, KN], dt.float32, tag="pss")
        nc.tensor.matmul(s1[:], lhsT=AT_sb[:], rhs=vb[:], start=True, stop=True)
        nc.scalar.copy(s1_sb[:], s1[:])
        s2 = pss.tile([1, KN], dt.float32, tag="pss")
        nc.tensor.matmul(s2[:], lhsT=ones64_sb[:], rhs=sqb[:], start=True, stop=True)
        rmsf = sbs.tile([1, KN], dt.float32, tag="rmsf")
        nc.scalar.activation(
            rmsf[:], s2[:], AF.Sqrt,
            bias=eps_sb[:1, :], scale=float(BOUND * BOUND / T),
        )
        fdr = io["f_dram"][ch:ch + 1].rearrange("a f -> (a f)")
        nc.scalar.dma_start(
            fdr[:(P + 2) * KN].rearrange("(a f) -> a f", a=P + 2), s1_sb[:]
        )
        nc.scalar.dma_start(
            fdr[(P + 2) * KN:].rearrange("(a f) -> a f", a=1), rmsf[:]
        )
        featsT = sbs.tile([FEAT, NC], dt.float32, tag="feats")
        nc.scalar.dma_start(
            featsT[:], fdr.rearrange("(jj k j) -> (jj k) j", k=K, j=NC)
        )
        featsT_r = sbs.tile([FEAT, NC], dt.float32r, tag="featsr")
        nc.vector.tensor_copy(featsT_r[:], featsT[:])
        y_sb = sby.tile([NC, D], dt.float32r, tag="y")
        for d0, dw in ((0, 512), (512, 256)):
            y_ps = pss.tile([NC, 512], dt.float32, tag="pss")
            nc.tensor.matmul(
                y_ps[:, :dw],
                lhsT=featsT_r[:],
                rhs=W_r[:, d0:d0 + dw],
                start=True,
                stop=True,
            )
            nc.scalar.copy(y_sb[:, d0:d0 + dw], y_ps[:, :dw])
        st["y"] = y_sb

    def phase3(ch, st):
        """y replicate via selector matmul + residual add + h out."""
        n0 = ch * NC
        x_parts, y_sb = st["x"], st["y"]
        h_parts = (sbh.tile([128, 4 * 768], dt.float32, tag="ha", name="ha"),
                   sbh.tile([128, 3 * 768], dt.float32, tag="hb", name="hb"))
        for pi, (ta, tb) in enumerate(((0, 4), (4, 7))):
            xp, hp = x_parts[pi], h_parts[pi]
            for i in range(tb - ta):
                tl = ta + i
                m_col = m_all[:, ch * NT + tl:ch * NT + tl + 1]
                for d0, dw in ((0, 512), (512, 256)):
                    ph = psh.tile([128, 512], dt.float32, tag="psh")
                    nc.tensor.matmul(
                        ph[:, :dw],
                        lhsT=repsel_r[:, tl * 128:(tl + 1) * 128],
                        rhs=y_sb[:, d0:d0 + dw],
                        start=True,
                        stop=True,
                    )
                    nc.vector.scalar_tensor_tensor(
                        hp[:, i * 768 + d0:i * 768 + d0 + dw],
                        in0=ph[:, :dw],
                        scalar=m_col,
                        in1=xp[:, i * 768 + d0:i * 768 + d0 + dw],
                        op0=ALU.mult,
                        op1=ALU.add,
                    )
            for half in range(2):
                nc.gpsimd.dma_start(
                    h_d[:, n0 + half + 2 * ta:n0 + half + 2 * tb - 1:2, :],
                    hp[half * 64:(half + 1) * 64, :].rearrange(
                        "p (tl d) -> p tl d", d=768
                    ),
                )

    # software-pipelined emission: phase1(it) | phase2(it-1) | phase3(it-2)
    total = NCH * reps
    states = {}
    for it in range(total + 3):
        if it >= 3:
            phase3((it - 3) % NCH, states[it - 3])
            del states[it - 3]
        if 1 <= it <= total:
            phase2((it - 1) % NCH, states[it - 1])
        if it < total:
            states[it] = phase1(it % NCH)


def build_program(reps=1):
    nc = bacc.Bacc("TRN2", target_bir_lowering=False, debug=False)
    io = {}
    io["x"] = nc.dram_tensor("x", [T, N, D], dt.float32, kind="ExternalInput").ap()
    io["U"] = nc.dram_tensor("U", [D, K], dt.float32, kind="ExternalInput").ap()
    io["ATc"] = nc.dram_tensor("ATc", [T, P + 2], dt.float32, kind="ExternalInput").ap()
    io["ones64"] = nc.dram_tensor("ones64", [T, 1], dt.float32, kind="ExternalInput").ap()
    io["eps128"] = nc.dram_tensor("eps128", [128, 1], dt.float32, kind="ExternalInput").ap()
    io["onesrow"] = nc.dram_tensor("onesrow", [1, T], dt.float32, kind="ExternalInput").ap()
    io["ident"] = nc.dram_tensor("ident", [128, 128], dt.float32, kind="ExternalInput").ap()
    io["repsel"] = nc.dram_tensor("repsel", [NC, NT * 128], dt.float32, kind="ExternalInput").ap()
    io["W_dev"] = nc.dram_tensor("W_dev", [FEAT, D], dt.float32, kind="ExternalInput").ap()
    io["m_all"] = nc.dram_tensor("m_all", [128, NCH * NT], dt.float32, kind="ExternalInput").ap()
    io["mask2"] = nc.dram_tensor("mask2", [T, NCH * KN], dt.float32, kind="ExternalInput").ap()
    io["h"] = nc.dram_tensor("h", [T, N, D], dt.float32, kind="ExternalOutput").ap()
    io["f_dram"] = nc.dram_tensor("f_dram", [NCH, (P + 3) * KN], dt.float32).ap()
    with tile.TileContext(nc) as tc:
        with ExitStack() as ctx:
            _emit(ctx, tc, io, reps=reps)
    nc.compile()
    return nc


def make_in_maps(x, valid_mask, proj_W, W_raw, beta):
    in_maps = []
    W_out = None
    for b in range(B):
        c = _host_consts(proj_W, W_raw, beta, np.asarray(valid_mask)[b])
        W_out = c["W_out"]
        in_maps.append({
            "x": np.ascontiguousarray(np.asarray(x)[b], dtype=np.float32),
            "U": c["U"], "ATc": c["ATc"], "ones64": c["ones64"], "eps128": c["eps128"],
            "onesrow": c["onesrow"], "ident": c["ident"],
            "repsel": c["repsel"], "W_dev": c["W_dev"],
            "m_all": c["m_all"], "mask2": c["mask2"],
        })
    return in_maps, W_out


_NC_CACHE = {}


def _get_program():
    if "nc" not in _NC_CACHE:
        _NC_CACHE["nc"] = build_program()
    return _NC_CACHE["nc"]


def run_hw(x, valid_mask, proj_W, W_raw, beta, trace=False, **kw):
    nc = _get_program()
    in_maps, W_out = make_in_maps(x, valid_mask, proj_W, W_raw, beta)
    res = run_bass_kernel_spmd(nc, in_maps, core_ids=list(range(B)), trace=trace, **kw)
    h = np.stack([res.results[b]["h"] for b in range(B)], axis=0)
    return h, W_out, res


def kernel(x, valid_mask, proj_W, W_raw, beta):
    h, W_out, _ = run_hw(x, valid_mask, proj_W, W_raw, beta, trace=False)
    return h, W_out
